# revision 21
# baseline (speedup 1.0000x reference)
"""Causal self-attention Trainium2 kernel (8-core head-parallel), v2.

Full inputs in, full output out. Sharding (per the head/tensor-parallel hint):
  - 16 heads / 8 cores -> 2 heads per core, both batch elems.
  - QKV column-parallel: per-core w_attn slice [1024, 384] (q|k|v 128 each),
    q pre-scaled by 1/sqrt(D).
  - c_proj row-parallel: per-core wp slice [128, 1024]; host sums the 8
    partial [B, C, T] outputs (the all-reduce), transposes, adds bias.

Key design points (1.4x over the 257us v1 baseline):
  - All-transposed on-device dataflow, zero activation transposes:
    xt [C, TOK] bf16 -> qkT [128 (h*64+d), 2, TOK], vT [128, TOK];
    V2 [128, 32, 2, 65] PE-transposed v-tiles with a ones column so the AV
    matmul emits the softmax denominator Z as output row 64 for free.
  - QW=512 q-windows; per s-tile step the two heads' K=64 S matmuls are
    emitted back-to-back into one [128, 2, 512] PSUM slab. Their lhsT base
    partitions (0/64) auto-derive tile_position row groups (0,0)/(64,0), so
    the pair runs CONCURRENTLY in the PE array (row tiling, ~2x S speedup;
    confirmed: second matmul of each pair retires in ~4ns).
  - One paired exp per step: a single strided ACT op [128, 2, w] over both
    heads' S psum banks, with bias -4 (cancels in normalization).
  - Causal mask folded into the S accumulation as a PE matmul that adds a
    constant strict-lower-triangular -30 block (ident^T @ trimask) on
    diagonal s-tiles -- no Vector/GpSimd op in the exp->AV chain.
  - Normalization off the critical path: 1/Z via DVE reciprocal_approx_fast
    (after a DVE copy to partition 0 -- custom DVE ops cannot cross
    partitions), GpSimd partition_broadcast to 64 rows, one DVE multiply
    that also drains the y psum into yT. Emitted as deferred closures
    drained inside the NEXT window's steps.
  - Global AV fifo (slack 8) lets consecutive windows flow into each other
    with no drain stall; a window's norm tails trigger when its last AV pops.
  - All remaining work (QKV for later token chunks, V transposes, c_proj
    tiles) is a dependency-ordered filler queue popped between attention
    steps, so the PE stays busy during exp waits. Window deps are
    force-emitted one window ahead.
  - DMA: per-line-dominated cost; whole [128, 512] descriptors, weights
    first, x token-chunk-major alternating b0/b1 (queues are FIFO, so
    issue order is priority). PSUM budget exactly 8 banks: S slab 2x2,
    y 2x1, misc (QKV/proj/transpose scratch) 2x1.
"""

import math
from collections import deque

import numpy as np
import ml_dtypes

import concourse.bass as bass
from concourse import bacc
import concourse.mybir as mybir
from concourse.tile import TileContext
from concourse.bass_utils import run_bass_kernel_spmd

BF16 = mybir.dt.bfloat16
F32 = mybir.dt.float32
NPBF16 = ml_dtypes.bfloat16

P = 128
B, T, C = 2, 2048, 1024
H, D = 16, 64
NCORES = 8
HPC = H // NCORES          # heads per core
TOK = B * T                # 4096 flattened tokens (b-major)
NCT = C // P               # 8 contraction tiles for the projections
QW = 512                   # q window width
NW = T // QW               # 4 windows per batch elem
EXP_BIAS = -4.0            # exp(s - 4): cancels in normalization, guards tail


def _patch_act_tables():
    """Force exp/ln onto the single table set containing both, avoiding
    mid-stream ACT_TABLE_LOAD switches."""
    import concourse.bacc as bacc_mod
    if getattr(bacc_mod, "_act_tables_patched", False):
        return
    orig = bacc_mod.get_activation_tables
    EXP = mybir.ActivationFunctionType.Exp
    LN = mybir.ActivationFunctionType.Ln

    def patched(arch):
        t = orig(arch)
        if any(EXP in f and LN in f for f in t.values()):
            for name, fns in t.items():
                if "natural_log_exp" not in name and (EXP in fns or LN in fns):
                    t[name] = fns - {EXP, LN}
        return t

    bacc_mod.get_activation_tables = patched
    bacc_mod._act_tables_patched = True


def build_nc(with_bias: bool) -> bacc.Bacc:
    _patch_act_tables()
    nc = bacc.Bacc(None, target_bir_lowering=False)

    xt = nc.dram_tensor("xt", [C, TOK], BF16, kind="ExternalInput")
    wqkv = nc.dram_tensor("wqkv", [C, 3 * P], BF16, kind="ExternalInput")
    wp = nc.dram_tensor("wp", [P, C], BF16, kind="ExternalInput")
    trimask = nc.dram_tensor("trimask", [P, P], BF16, kind="ExternalInput")
    ident = nc.dram_tensor("ident", [P, P], BF16, kind="ExternalInput")
    if with_bias:
        bqkv = nc.dram_tensor("bqkv", [1, 3 * P], BF16, kind="ExternalInput")
        ones512 = nc.dram_tensor("ones512", [1, 512], BF16, kind="ExternalInput")
    outT = nc.dram_tensor("outT", [B, C, T], BF16, kind="ExternalOutput")

    EXP = mybir.ActivationFunctionType.Exp

    with TileContext(nc) as tc:
        with (
            tc.tile_pool(name="consts", bufs=1) as consts,
            tc.tile_pool(name="px", bufs=1) as px,
            tc.tile_pool(name="pqkv", bufs=1) as pqkv,
            tc.tile_pool(name="py", bufs=1) as py,
            tc.tile_pool(name="sbw", bufs=1) as sbw,
            tc.tile_pool(name="ps_sp", bufs=2, space="PSUM") as ps_sp,
            tc.tile_pool(name="ps_y", bufs=2, space="PSUM") as ps_y,
            tc.tile_pool(name="ps_misc", bufs=2, space="PSUM") as ps_misc,
        ):
            # ---- constant loads (small, land first) ----
            trimask_sb = consts.tile([P, P], BF16)
            nc.sync.dma_start(trimask_sb, trimask[:, :])
            ident_sb = consts.tile([P, P], BF16)
            nc.sync.dma_start(ident_sb, ident[:, :])
            # DMA cost is per-line dominated: keep whole [128, 512]
            # descriptors; wqkv first (queues are FIFO), then x token-major.
            wqkv_sb = consts.tile([P, NCT, 3 * P], BF16)
            for ct in range(NCT):
                nc.sync.dma_start(wqkv_sb[:, ct, :], wqkv[ct * P:(ct + 1) * P, :])
            wp_sb = consts.tile([P, C], BF16)
            nc.sync.dma_start(wp_sb, wp[:, :])
            expb = consts.tile([P, 1], F32)
            nc.vector.memset(expb, EXP_BIAS)
            if with_bias:
                bqkv_sb = consts.tile([1, 3 * P], BF16)
                nc.sync.dma_start(bqkv_sb, bqkv[:, :])
                ones512_sb = consts.tile([1, 512], BF16)
                nc.sync.dma_start(ones512_sb, ones512[:, :])
            xt_sb = px.tile([P, NCT, TOK], BF16)
            for tcc in (0, 4, 1, 5, 2, 6, 3, 7):
                for ct in range(NCT):
                    nc.sync.dma_start(
                        xt_sb[:, ct, tcc * 512:(tcc + 1) * 512],
                        xt[ct * P:(ct + 1) * P, tcc * 512:(tcc + 1) * 512])

            # big SBUF slabs
            qkT = pqkv.tile([P, 2, TOK], BF16)      # rows h*64+d; dim1 0=q 1=k
            vT = pqkv.tile([P, TOK], BF16)
            V2 = pqkv.tile([P, TOK // P, HPC, 65], BF16)
            nc.vector.memset(V2, 1.0)               # col 64 = ones column
            yT = py.tile([P, B, T], BF16)

            # ---- unit emitters ----
            copy_eng = [0]  # alternate ot psum->sbuf copies DVE/ACT

            def copy_out(dst, src, alternate=False):
                copy_eng[0] ^= 1
                if not alternate or copy_eng[0]:
                    nc.vector.tensor_copy(dst, src)
                else:
                    nc.scalar.copy(dst, src)

            qkv_pending = {}

            def emit_qkv_half(tcc, ft, half):
                """First half allocates the psum tile and does ct 0-3; the
                second half finishes ct 4-7 (+bias) and drains. Split so the
                filler granularity stays ~1us of PE work."""
                if half == 0:
                    pq = ps_misc.tile([P, 512], F32, tag="misc",
                                      name=f"pq_{tcc}_{ft}")
                    qkv_pending[(tcc, ft)] = pq
                    cts = range(0, NCT // 2)
                else:
                    pq = qkv_pending.pop((tcc, ft))
                    cts = range(NCT // 2, NCT)
                for ct in cts:
                    nc.tensor.matmul(
                        pq,
                        wqkv_sb[:, ct, ft * P:(ft + 1) * P],
                        xt_sb[:, ct, tcc * 512:(tcc + 1) * 512],
                        start=(ct == 0),
                        stop=(ct == NCT - 1 and not with_bias),
                    )
                if half == 0:
                    return
                if with_bias:
                    nc.tensor.matmul(
                        pq,
                        bqkv_sb[0:1, ft * P:(ft + 1) * P],
                        ones512_sb[0:1, :],
                        start=False, stop=True,
                    )
                if ft < 2:
                    copy_out(qkT[:, ft, tcc * 512:(tcc + 1) * 512], pq)
                else:
                    copy_out(vT[:, tcc * 512:(tcc + 1) * 512], pq)

            def emit_qkv_unit(tcc, ft):
                emit_qkv_half(tcc, ft, 0)
                emit_qkv_half(tcc, ft, 1)

            def emit_vtrans_unit(sg):  # sg = global s-tile 0..31
                pt = ps_misc.tile([P, P], BF16, tag="misc", name=f"pt_{sg}")
                nc.tensor.transpose(
                    pt, vT[:, sg * P:(sg + 1) * P], ident_sb)
                # cols 0:64 -> head0 d, 64:128 -> head1 d, in one copy
                nc.vector.tensor_copy(
                    V2[:, sg, :, 0:64],
                    pt[:, :].rearrange("p (h d) -> p h d", h=2),
                )

            def emit_proj_unit(b, qh, of, split=False):
                po = ps_misc.tile([P, 512], F32, tag="misc",
                                  name=f"po_{b}_{qh}_{of}")
                nc.tensor.matmul(
                    po,
                    wp_sb[:, of * P:(of + 1) * P],
                    yT[:, b, qh * QW:(qh + 1) * QW],
                    start=True, stop=True,
                )
                ot = sbw.tile([P, 512], BF16, tag="ot", bufs=6,
                              name=f"ot_{b}_{qh}_{of}")
                copy_out(ot, po)
                if split:
                    # tail drain: halve lines per descriptor, use 2 queues
                    for hp in range(2):
                        nc.sync.dma_start(
                            outT[b, of * P + hp * 64:of * P + (hp + 1) * 64,
                                 qh * QW:(qh + 1) * QW],
                            ot[hp * 64:(hp + 1) * 64, :])
                else:
                    nc.sync.dma_start(
                        outT[b, of * P:(of + 1) * P,
                             qh * QW:(qh + 1) * QW], ot)

            filler = deque()

            def pop_filler(n):
                for _ in range(min(n, len(filler))):
                    f, *a = filler.popleft()
                    f(*a)

            def force_units(pred):
                """Emit every queued unit matching pred (dependency order is
                preserved because filler is popped front-first)."""
                keep = deque()
                while filler:
                    item = filler.popleft()
                    if pred(item):
                        f, *a = item
                        f(*a)
                    else:
                        keep.append(item)
                filler.extend(keep)

            # ---- attention window ----
            def make_norm_tails(b, qh, ys):
                """Normalization chain for a finished window, returned as
                closures drained inside the NEXT window's steps (keeps the
                ACT->DVE->Pool->DVE chain latency off the critical path)."""
                qbase = qh * QW
                state = {}

                def t_recip():
                    for h in range(HPC):
                        zrow = sbw.tile([1, QW], F32, tag="zrow", bufs=4,
                                        name=f"zrow_{b}_{qh}_{h}")
                        # moves Z from psum partition 64 to partition 0
                        # (custom DVE ops cannot cross partitions)
                        nc.vector.tensor_copy(zrow, ys[h][64:65, 0:QW])
                        rr = sbw.tile([1, QW], F32, tag="rr", bufs=4,
                                      name=f"rr_{b}_{qh}_{h}")
                        nc.vector.reciprocal_approx_fast(rr, zrow)
                        state[h] = rr

                def t_bcast():
                    for h in range(HPC):
                        rbh = sbw.tile([64, QW], F32, tag="rb", bufs=4,
                                       name=f"rb_{b}_{qh}_{h}")
                        nc.gpsimd.partition_broadcast(rbh, state[h],
                                                      channels=64)
                        state[h] = rbh

                def t_mult():
                    for h in range(HPC):
                        nc.vector.tensor_mul(
                            yT[h * 64:(h + 1) * 64, b, qbase:qbase + QW],
                            ys[h][0:64, 0:QW],
                            state[h],
                        )
                    last = (b, qh) == (B - 1, NW - 1)
                    for of in range(NCT):
                        filler.append((emit_proj_unit, b, qh, of, last))

                return [t_recip, t_bcast, t_mult]

            pending_tails = []
            av_fifo = deque()   # global: windows flow into each other

            def pop_av(n):
                for _ in range(min(n, len(av_fifo))):
                    f, is_last, tail_fn = av_fifo.popleft()
                    f()
                    if is_last:
                        pending_tails.extend(tail_fn())

            def window(b, qh):
                qbase = qh * QW
                n_st = (qbase + QW) // P
                y0 = ps_y.tile([65, QW], F32, tag="y", name=f"y0_{b}_{qh}")
                y1 = ps_y.tile([65, QW], F32, tag="y", name=f"y1_{b}_{qh}")
                ys = (y0, y1)
                tail_fn = lambda: make_norm_tails(b, qh, ys)
                for st in range(n_st):
                    s0 = st * P
                    qa = max(qbase, s0)
                    off = qa - qbase
                    w = QW - off
                    diag = s0 >= qbase
                    sp = ps_sp.tile([P, HPC, QW], F32, tag="sp",
                                    name=f"sp_{b}_{qh}_{st}")
                    for h in range(HPC):
                        nc.tensor.matmul(
                            sp[:, h, 0:w],
                            qkT[64 * h:64 * h + 64, 1,
                                b * T + s0:b * T + s0 + P],
                            qkT[64 * h:64 * h + 64, 0,
                                b * T + qa:b * T + qa + w],
                            start=True, stop=not diag,
                        )
                    if diag:
                        # causal mask folded into S: add -30 on j < s_local
                        for h in range(HPC):
                            nc.tensor.matmul(
                                sp[:, h, 0:P],
                                ident_sb,
                                trimask_sb,
                                start=False, stop=True,
                            )
                    es = sbw.tile([P, HPC, QW], BF16, tag="es", bufs=10,
                                  name=f"es_{b}_{qh}_{st}")
                    nc.scalar.activation(
                        es[:, :, 0:w], sp[:, :, 0:w], EXP, bias=expb)
                    for h in range(HPC):
                        def av(h=h, es=es, off=off, w=w, st=st, ys=ys, b=b,
                               n_st=n_st):
                            nc.tensor.matmul(
                                ys[h][0:65, off:off + w],
                                V2[:, b * (T // P) + st, h, :],
                                es[:, h, 0:w],
                                start=(st == 0),
                                stop=(st == n_st - 1),
                            )
                        av_fifo.append(
                            (av, st == n_st - 1 and h == HPC - 1, tail_fn))
                    pop_av(len(av_fifo) - 8)
                    if pending_tails:
                        pending_tails.pop(0)()
                        pop_filler(1 if len(filler) > 8 else 0)
                    elif st % 2 == 0:
                        pop_filler(3 if len(filler) > 8 else 1)

            # preload the exp table set during the initial DMA wait
            tbl_warm = consts.tile([1, 1], F32)
            nc.scalar.activation(tbl_warm, expb[0:1, 0:1], EXP)

            # ---- HAM warm-up while first DMAs land ----
            scratch = ps_misc.tile([P, 512], F32, tag="misc")
            for _ in range(4):
                nc.tensor.matmul(
                    scratch[:, 0:P], ident_sb, ident_sb,
                    start=True, stop=True,
                )
            for _ in range(8):
                nc.tensor.matmul(
                    scratch[:, 0:384],
                    wqkv_sb[:, 0, 0:P],
                    wqkv_sb[:, 0, :],
                    start=True, stop=True,
                )

            # ---- static schedule ----
            # phase A: QKV + v-transposes for token chunk 0 (tokens 0:512)
            for ft in (0, 1, 2):
                emit_qkv_unit(0, ft)
            for sg in range(4):
                emit_vtrans_unit(sg)

            # filler queue in dependency-safe order
            for tcc in (4, 1, 5, 2, 6, 3, 7):
                for ft in (2, 0, 1):
                    filler.append((emit_qkv_unit, tcc, ft))
                for sg in range(4 * tcc, 4 * tcc + 4):
                    filler.append((emit_vtrans_unit, sg))

            def need_for(b, qh):
                """Units that must be emitted before window (b, qh)."""
                tcc_max = b * 4 + qh
                sg_max = b * (T // P) + (qh + 1) * 4 - 1

                def pred(item):
                    f = item[0]
                    if f is emit_qkv_unit:
                        return item[1] <= tcc_max
                    if f is emit_vtrans_unit:
                        return item[1] <= sg_max
                    return False
                return pred

            wins = [(b, qh) for qh in range(NW) for b in range(B)]
            for i, (b, qh) in enumerate(wins):
                force_units(need_for(b, qh))
                if i + 1 < len(wins):
                    force_units(need_for(*wins[i + 1]))
                window(b, qh)
            pop_av(len(av_fifo))
            for t in pending_tails:
                t()
            pop_filler(len(filler))
    nc.compile()
    return nc


_CACHE = {}


def _get_nc(with_bias: bool) -> bacc.Bacc:
    if with_bias not in _CACHE:
        _CACHE[with_bias] = build_nc(with_bias)
    return _CACHE[with_bias]


def _prep_inputs(x, w_attn, b_attn, w_proj):
    """Host-side shard + layout prep. Returns per-core in_maps."""
    xf = np.ascontiguousarray(
        np.asarray(x, dtype=np.float32).reshape(TOK, C).T
    ).astype(NPBF16)                                   # x^T [C, TOK]
    w = np.asarray(w_attn, dtype=np.float32)
    ba = np.asarray(b_attn, dtype=np.float32)
    wpj = np.asarray(w_proj, dtype=np.float32)
    scale = 1.0 / math.sqrt(D)
    with_bias = bool(np.any(ba))

    trimask_np = np.tril(
        np.full((P, P), -30.0, dtype=np.float32), -1).astype(NPBF16)
    id_np = np.eye(P, dtype=np.float32).astype(NPBF16)
    ones512_np = np.ones((1, 512), dtype=np.float32).astype(NPBF16)

    in_maps = []
    for c in range(NCORES):
        lo, hi = c * HPC * D, (c + 1) * HPC * D        # 128-wide head slice
        wq = w[:, lo:hi] * scale
        wk = w[:, C + lo:C + hi]
        wv = w[:, 2 * C + lo:2 * C + hi]
        wqkv_c = np.concatenate([wq, wk, wv], axis=1).astype(NPBF16)
        wp_c = np.ascontiguousarray(wpj[lo:hi, :]).astype(NPBF16)
        m = {
            "xt": xf,
            "wqkv": wqkv_c,
            "wp": wp_c,
            "trimask": trimask_np,
            "ident": id_np,
        }
        if with_bias:
            bq = ba[lo:hi] * scale
            bk = ba[C + lo:C + hi]
            bv = ba[2 * C + lo:2 * C + hi]
            m["bqkv"] = np.concatenate([bq, bk, bv])[None, :].astype(NPBF16)
            m["ones512"] = ones512_np
        in_maps.append(m)
    return in_maps, with_bias


def _combine(results, b_proj):
    acc = np.zeros((B, C, T), dtype=np.float32)
    for r in results:
        acc += np.asarray(r["outT"], dtype=np.float32)
    out = np.transpose(acc, (0, 2, 1))                 # [B, T, C]
    out = out + np.asarray(b_proj, dtype=np.float32)[None, None, :]
    return np.ascontiguousarray(out.astype(np.float32))


def run(x, w_attn, b_attn, w_proj, b_proj, trace=False, trace_cores=None):
    in_maps, with_bias = _prep_inputs(x, w_attn, b_attn, w_proj)
    nc = _get_nc(with_bias)
    res = run_bass_kernel_spmd(
        nc, in_maps, core_ids=list(range(NCORES)),
        trace=trace, trace_cores=trace_cores,
    )
    return _combine(res.results, b_proj), res


def kernel(x, w_attn, b_attn, w_proj, b_proj):
    out, _ = run(x, w_attn, b_attn, w_proj, b_proj, trace=False)
    return out


# revision 22
# speedup vs baseline: 1.0253x; 1.0253x over previous
"""Causal self-attention Trainium2 kernel (8-core head-parallel), v2.

Full inputs in, full output out. Sharding (per the head/tensor-parallel hint):
  - 16 heads / 8 cores -> 2 heads per core, both batch elems.
  - QKV column-parallel: per-core w_attn slice [1024, 384] (q|k|v 128 each),
    q pre-scaled by 1/sqrt(D).
  - c_proj row-parallel: per-core wp slice [128, 1024]; host sums the 8
    partial [B, C, T] outputs (the all-reduce), transposes, adds bias.

Key design points (1.4x over the 257us v1 baseline):
  - All-transposed on-device dataflow, zero activation transposes:
    xt [C, TOK] bf16 -> qkT [128 (h*64+d), 2, TOK], vT [128, TOK];
    V2 [128, 32, 2, 65] PE-transposed v-tiles with a ones column so the AV
    matmul emits the softmax denominator Z as output row 64 for free.
  - QW=512 q-windows; per s-tile step the two heads' K=64 S matmuls are
    emitted back-to-back into one [128, 2, 512] PSUM slab. Their lhsT base
    partitions (0/64) auto-derive tile_position row groups (0,0)/(64,0), so
    the pair runs CONCURRENTLY in the PE array (row tiling, ~2x S speedup;
    confirmed: second matmul of each pair retires in ~4ns).
  - One paired exp per step: a single strided ACT op [128, 2, w] over both
    heads' S psum banks, with bias -4 (cancels in normalization).
  - Causal mask folded into the S accumulation as a PE matmul that adds a
    constant strict-lower-triangular -30 block (ident^T @ trimask) on
    diagonal s-tiles -- no Vector/GpSimd op in the exp->AV chain.
  - Normalization off the critical path: 1/Z via DVE reciprocal_approx_fast
    (after a DVE copy to partition 0 -- custom DVE ops cannot cross
    partitions), GpSimd partition_broadcast to 64 rows, one DVE multiply
    that also drains the y psum into yT. Emitted as deferred closures
    drained inside the NEXT window's steps.
  - Global AV fifo (slack 8) lets consecutive windows flow into each other
    with no drain stall; a window's norm tails trigger when its last AV pops.
  - All remaining work (QKV for later token chunks, V transposes, c_proj
    tiles) is a dependency-ordered filler queue popped between attention
    steps, so the PE stays busy during exp waits. Window deps are
    force-emitted one window ahead.
  - DMA: per-line-dominated cost; whole [128, 512] descriptors, weights
    first, x token-chunk-major alternating b0/b1 (queues are FIFO, so
    issue order is priority). PSUM budget exactly 8 banks: S slab 2x2,
    y 2x1, misc (QKV/proj/transpose scratch) 2x1.
"""

import math
from collections import deque

import numpy as np
import ml_dtypes

import concourse.bass as bass
from concourse import bacc
import concourse.mybir as mybir
from concourse.tile import TileContext
from concourse.bass_utils import run_bass_kernel_spmd

BF16 = mybir.dt.bfloat16
F32 = mybir.dt.float32
NPBF16 = ml_dtypes.bfloat16

P = 128
B, T, C = 2, 2048, 1024
H, D = 16, 64
NCORES = 8
HPC = H // NCORES          # heads per core
TOK = B * T                # 4096 flattened tokens (b-major)
NCT = C // P               # 8 contraction tiles for the projections
QW = 512                   # q window width
NW = T // QW               # 4 windows per batch elem
EXP_BIAS = -4.0            # exp(s - 4): cancels in normalization, guards tail


def _patch_act_tables():
    """Force exp/ln onto the single table set containing both, avoiding
    mid-stream ACT_TABLE_LOAD switches."""
    import concourse.bacc as bacc_mod
    if getattr(bacc_mod, "_act_tables_patched", False):
        return
    orig = bacc_mod.get_activation_tables
    EXP = mybir.ActivationFunctionType.Exp
    LN = mybir.ActivationFunctionType.Ln

    def patched(arch):
        t = orig(arch)
        if any(EXP in f and LN in f for f in t.values()):
            for name, fns in t.items():
                if "natural_log_exp" not in name and (EXP in fns or LN in fns):
                    t[name] = fns - {EXP, LN}
        return t

    bacc_mod.get_activation_tables = patched
    bacc_mod._act_tables_patched = True


def build_nc(with_bias: bool) -> bacc.Bacc:
    _patch_act_tables()
    nc = bacc.Bacc(None, target_bir_lowering=False)

    xt = nc.dram_tensor("xt", [C, TOK], BF16, kind="ExternalInput")
    wqkv = nc.dram_tensor("wqkv", [C, 3 * P], BF16, kind="ExternalInput")
    wp = nc.dram_tensor("wp", [P, C], BF16, kind="ExternalInput")
    trimask = nc.dram_tensor("trimask", [P, P], BF16, kind="ExternalInput")
    ident = nc.dram_tensor("ident", [P, P], BF16, kind="ExternalInput")
    if with_bias:
        bqkv = nc.dram_tensor("bqkv", [1, 3 * P], BF16, kind="ExternalInput")
        ones512 = nc.dram_tensor("ones512", [1, 512], BF16, kind="ExternalInput")
    outT = nc.dram_tensor("outT", [B, C, T], BF16, kind="ExternalOutput")

    EXP = mybir.ActivationFunctionType.Exp

    with TileContext(nc) as tc:
        with (
            tc.tile_pool(name="consts", bufs=1) as consts,
            tc.tile_pool(name="px", bufs=1) as px,
            tc.tile_pool(name="pqkv", bufs=1) as pqkv,
            tc.tile_pool(name="py", bufs=1) as py,
            tc.tile_pool(name="sbw", bufs=1) as sbw,
            tc.tile_pool(name="ps_sp", bufs=2, space="PSUM") as ps_sp,
            tc.tile_pool(name="ps_y", bufs=2, space="PSUM") as ps_y,
            tc.tile_pool(name="ps_misc", bufs=2, space="PSUM") as ps_misc,
        ):
            # ---- constant loads (small, land first) ----
            trimask_sb = consts.tile([P, P], BF16)
            nc.sync.dma_start(trimask_sb, trimask[:, :])
            ident_sb = consts.tile([P, P], BF16)
            nc.sync.dma_start(ident_sb, ident[:, :])
            # DMA cost is per-line dominated: keep whole [128, 512]
            # descriptors; wqkv first (queues are FIFO), then x token-major.
            wqkv_sb = consts.tile([P, NCT, 3 * P], BF16)
            for ct in range(NCT):
                nc.sync.dma_start(wqkv_sb[:, ct, :], wqkv[ct * P:(ct + 1) * P, :])
            wp_sb = consts.tile([P, C], BF16)
            nc.sync.dma_start(wp_sb, wp[:, :])
            expb = consts.tile([P, 1], F32)
            nc.vector.memset(expb, EXP_BIAS)
            if with_bias:
                bqkv_sb = consts.tile([1, 3 * P], BF16)
                nc.sync.dma_start(bqkv_sb, bqkv[:, :])
                ones512_sb = consts.tile([1, 512], BF16)
                nc.sync.dma_start(ones512_sb, ones512[:, :])
            xt_sb = px.tile([P, NCT, TOK], BF16)
            for tcc in (0, 4, 1, 5, 2, 6, 3, 7):
                for ct in range(NCT):
                    nc.sync.dma_start(
                        xt_sb[:, ct, tcc * 512:(tcc + 1) * 512],
                        xt[ct * P:(ct + 1) * P, tcc * 512:(tcc + 1) * 512])

            # big SBUF slabs
            qkT = pqkv.tile([P, 2, TOK], BF16)      # rows h*64+d; dim1 0=q 1=k
            vT = pqkv.tile([P, TOK], BF16)
            V2 = pqkv.tile([P, TOK // P, HPC, 65], BF16)
            nc.vector.memset(V2, 1.0)               # col 64 = ones column
            yT = py.tile([P, B, T], BF16)

            # ---- unit emitters ----
            copy_eng = [0]  # alternate ot psum->sbuf copies DVE/ACT

            def copy_out(dst, src, alternate=False):
                copy_eng[0] ^= 1
                if not alternate or copy_eng[0]:
                    nc.vector.tensor_copy(dst, src)
                else:
                    nc.scalar.copy(dst, src)

            qkv_pending = {}

            def emit_qkv_half(tcc, ft, half):
                """First half allocates the psum tile and does ct 0-3; the
                second half finishes ct 4-7 (+bias) and drains. Split so the
                filler granularity stays ~1us of PE work."""
                if half == 0:
                    pq = ps_misc.tile([P, 512], F32, tag="misc",
                                      name=f"pq_{tcc}_{ft}")
                    qkv_pending[(tcc, ft)] = pq
                    cts = range(0, NCT // 2)
                else:
                    pq = qkv_pending.pop((tcc, ft))
                    cts = range(NCT // 2, NCT)
                for ct in cts:
                    nc.tensor.matmul(
                        pq,
                        wqkv_sb[:, ct, ft * P:(ft + 1) * P],
                        xt_sb[:, ct, tcc * 512:(tcc + 1) * 512],
                        start=(ct == 0),
                        stop=(ct == NCT - 1 and not with_bias),
                    )
                if half == 0:
                    return
                if with_bias:
                    nc.tensor.matmul(
                        pq,
                        bqkv_sb[0:1, ft * P:(ft + 1) * P],
                        ones512_sb[0:1, :],
                        start=False, stop=True,
                    )
                if ft < 2:
                    copy_out(qkT[:, ft, tcc * 512:(tcc + 1) * 512], pq)
                else:
                    copy_out(vT[:, tcc * 512:(tcc + 1) * 512], pq)

            def emit_qkv_unit(tcc, ft):
                emit_qkv_half(tcc, ft, 0)
                emit_qkv_half(tcc, ft, 1)

            def emit_vtrans_unit(sg):  # sg = global s-tile 0..31
                pt = ps_misc.tile([P, P], BF16, tag="misc", name=f"pt_{sg}")
                nc.tensor.transpose(
                    pt, vT[:, sg * P:(sg + 1) * P], ident_sb)
                # cols 0:64 -> head0 d, 64:128 -> head1 d, in one copy
                nc.vector.tensor_copy(
                    V2[:, sg, :, 0:64],
                    pt[:, :].rearrange("p (h d) -> p h d", h=2),
                )

            def emit_proj_unit(b, qh, of, split=False):
                po = ps_misc.tile([P, 512], F32, tag="misc",
                                  name=f"po_{b}_{qh}_{of}")
                nc.tensor.matmul(
                    po,
                    wp_sb[:, of * P:(of + 1) * P],
                    yT[:, b, qh * QW:(qh + 1) * QW],
                    start=True, stop=True,
                )
                ot = sbw.tile([P, 512], BF16, tag="ot", bufs=6,
                              name=f"ot_{b}_{qh}_{of}")
                copy_out(ot, po)
                if split:
                    # tail drain: halve lines per descriptor, use 2 queues
                    for hp in range(2):
                        nc.sync.dma_start(
                            outT[b, of * P + hp * 64:of * P + (hp + 1) * 64,
                                 qh * QW:(qh + 1) * QW],
                            ot[hp * 64:(hp + 1) * 64, :])
                else:
                    nc.sync.dma_start(
                        outT[b, of * P:(of + 1) * P,
                             qh * QW:(qh + 1) * QW], ot)

            filler = deque()

            def pop_filler(n):
                for _ in range(min(n, len(filler))):
                    f, *a = filler.popleft()
                    f(*a)

            def force_units(pred):
                """Emit every queued unit matching pred (dependency order is
                preserved because filler is popped front-first)."""
                keep = deque()
                while filler:
                    item = filler.popleft()
                    if pred(item):
                        f, *a = item
                        f(*a)
                    else:
                        keep.append(item)
                filler.extend(keep)

            # ---- attention window ----
            def make_norm_tails(b, qh, ys):
                """Normalization chain for a finished window, returned as
                closures drained inside the NEXT window's steps (keeps the
                ACT->DVE->Pool->DVE chain latency off the critical path)."""
                qbase = qh * QW
                state = {}

                def t_recip():
                    for h in range(HPC):
                        zrow = sbw.tile([1, QW], F32, tag="zrow", bufs=4,
                                        name=f"zrow_{b}_{qh}_{h}")
                        # moves Z from psum partition 64 to partition 0
                        # (custom DVE ops cannot cross partitions)
                        nc.vector.tensor_copy(zrow, ys[h][64:65, 0:QW])
                        rr = sbw.tile([1, QW], F32, tag="rr", bufs=4,
                                      name=f"rr_{b}_{qh}_{h}")
                        nc.vector.reciprocal_approx_fast(rr, zrow)
                        state[h] = rr

                def t_bcast():
                    for h in range(HPC):
                        rbh = sbw.tile([64, QW], F32, tag="rb", bufs=4,
                                       name=f"rb_{b}_{qh}_{h}")
                        nc.gpsimd.partition_broadcast(rbh, state[h],
                                                      channels=64)
                        state[h] = rbh

                def t_mult():
                    for h in range(HPC):
                        nc.vector.tensor_mul(
                            yT[h * 64:(h + 1) * 64, b, qbase:qbase + QW],
                            ys[h][0:64, 0:QW],
                            state[h],
                        )
                    last = (b, qh) == (B - 1, NW - 1)
                    for of in range(NCT):
                        filler.append((emit_proj_unit, b, qh, of, last))

                return [t_recip, t_bcast, t_mult]

            pending_tails = []
            av_fifo = deque()   # global: windows flow into each other

            def pop_av(n):
                for _ in range(min(n, len(av_fifo))):
                    f, is_last, tail_fn = av_fifo.popleft()
                    f()
                    if is_last:
                        pending_tails.extend(tail_fn())

            def window(b, qh):
                qbase = qh * QW
                n_st = (qbase + QW) // P
                y0 = ps_y.tile([65, QW], F32, tag="y", name=f"y0_{b}_{qh}")
                y1 = ps_y.tile([65, QW], F32, tag="y", name=f"y1_{b}_{qh}")
                ys = (y0, y1)
                tail_fn = lambda: make_norm_tails(b, qh, ys)
                for st in range(n_st):
                    s0 = st * P
                    qa = max(qbase, s0)
                    off = qa - qbase
                    w = QW - off
                    diag = s0 >= qbase
                    sp = ps_sp.tile([P, HPC, QW], F32, tag="sp",
                                    name=f"sp_{b}_{qh}_{st}")
                    for h in range(HPC):
                        nc.tensor.matmul(
                            sp[:, h, 0:w],
                            qkT[64 * h:64 * h + 64, 1,
                                b * T + s0:b * T + s0 + P],
                            qkT[64 * h:64 * h + 64, 0,
                                b * T + qa:b * T + qa + w],
                            start=True, stop=not diag,
                        )
                    if diag:
                        # causal mask folded into S: add -30 on j < s_local
                        for h in range(HPC):
                            nc.tensor.matmul(
                                sp[:, h, 0:P],
                                ident_sb,
                                trimask_sb,
                                start=False, stop=True,
                            )
                    es = sbw.tile([P, HPC, QW], BF16, tag="es", bufs=12,
                                  name=f"es_{b}_{qh}_{st}")
                    nc.scalar.activation(
                        es[:, :, 0:w], sp[:, :, 0:w], EXP, bias=expb)
                    for h in range(HPC):
                        def av(h=h, es=es, off=off, w=w, st=st, ys=ys, b=b,
                               n_st=n_st):
                            nc.tensor.matmul(
                                ys[h][0:65, off:off + w],
                                V2[:, b * (T // P) + st, h, :],
                                es[:, h, 0:w],
                                start=(st == 0),
                                stop=(st == n_st - 1),
                            )
                        av_fifo.append(
                            (av, st == n_st - 1 and h == HPC - 1, tail_fn))
                    if len(av_fifo) >= 10:
                        pop_av(4)
                    if pending_tails:
                        pending_tails.pop(0)()
                        pop_filler(1 if len(filler) > 8 else 0)
                    elif st % 2 == 0:
                        pop_filler(3 if len(filler) > 8 else 1)

            # preload the exp table set during the initial DMA wait
            tbl_warm = consts.tile([1, 1], F32)
            nc.scalar.activation(tbl_warm, expb[0:1, 0:1], EXP)

            # ---- HAM warm-up while first DMAs land ----
            scratch = ps_misc.tile([P, 512], F32, tag="misc")
            for _ in range(4):
                nc.tensor.matmul(
                    scratch[:, 0:P], ident_sb, ident_sb,
                    start=True, stop=True,
                )
            for _ in range(8):
                nc.tensor.matmul(
                    scratch[:, 0:384],
                    wqkv_sb[:, 0, 0:P],
                    wqkv_sb[:, 0, :],
                    start=True, stop=True,
                )

            # ---- static schedule ----
            # phase A: QKV + v-transposes for token chunk 0 (tokens 0:512)
            for ft in (0, 1, 2):
                emit_qkv_unit(0, ft)
            for sg in range(4):
                emit_vtrans_unit(sg)

            # filler queue in dependency-safe order
            for tcc in (4, 1, 5, 2, 6, 3, 7):
                for ft in (2, 0, 1):
                    filler.append((emit_qkv_unit, tcc, ft))
                for sg in range(4 * tcc, 4 * tcc + 4):
                    filler.append((emit_vtrans_unit, sg))

            def need_for(b, qh):
                """Units that must be emitted before window (b, qh)."""
                tcc_max = b * 4 + qh
                sg_max = b * (T // P) + (qh + 1) * 4 - 1

                def pred(item):
                    f = item[0]
                    if f is emit_qkv_unit:
                        return item[1] <= tcc_max
                    if f is emit_vtrans_unit:
                        return item[1] <= sg_max
                    return False
                return pred

            wins = [(b, qh) for qh in range(NW) for b in range(B)]
            for i, (b, qh) in enumerate(wins):
                force_units(need_for(b, qh))
                if i + 1 < len(wins):
                    force_units(need_for(*wins[i + 1]))
                window(b, qh)
            pop_av(len(av_fifo))
            for t in pending_tails:
                t()
            pop_filler(len(filler))
    nc.compile()
    return nc


_CACHE = {}


def _get_nc(with_bias: bool) -> bacc.Bacc:
    if with_bias not in _CACHE:
        _CACHE[with_bias] = build_nc(with_bias)
    return _CACHE[with_bias]


def _prep_inputs(x, w_attn, b_attn, w_proj):
    """Host-side shard + layout prep. Returns per-core in_maps."""
    xf = np.ascontiguousarray(
        np.asarray(x, dtype=np.float32).reshape(TOK, C).T
    ).astype(NPBF16)                                   # x^T [C, TOK]
    w = np.asarray(w_attn, dtype=np.float32)
    ba = np.asarray(b_attn, dtype=np.float32)
    wpj = np.asarray(w_proj, dtype=np.float32)
    scale = 1.0 / math.sqrt(D)
    with_bias = bool(np.any(ba))

    trimask_np = np.tril(
        np.full((P, P), -30.0, dtype=np.float32), -1).astype(NPBF16)
    id_np = np.eye(P, dtype=np.float32).astype(NPBF16)
    ones512_np = np.ones((1, 512), dtype=np.float32).astype(NPBF16)

    in_maps = []
    for c in range(NCORES):
        lo, hi = c * HPC * D, (c + 1) * HPC * D        # 128-wide head slice
        wq = w[:, lo:hi] * scale
        wk = w[:, C + lo:C + hi]
        wv = w[:, 2 * C + lo:2 * C + hi]
        wqkv_c = np.concatenate([wq, wk, wv], axis=1).astype(NPBF16)
        wp_c = np.ascontiguousarray(wpj[lo:hi, :]).astype(NPBF16)
        m = {
            "xt": xf,
            "wqkv": wqkv_c,
            "wp": wp_c,
            "trimask": trimask_np,
            "ident": id_np,
        }
        if with_bias:
            bq = ba[lo:hi] * scale
            bk = ba[C + lo:C + hi]
            bv = ba[2 * C + lo:2 * C + hi]
            m["bqkv"] = np.concatenate([bq, bk, bv])[None, :].astype(NPBF16)
            m["ones512"] = ones512_np
        in_maps.append(m)
    return in_maps, with_bias


def _combine(results, b_proj):
    acc = np.zeros((B, C, T), dtype=np.float32)
    for r in results:
        acc += np.asarray(r["outT"], dtype=np.float32)
    out = np.transpose(acc, (0, 2, 1))                 # [B, T, C]
    out = out + np.asarray(b_proj, dtype=np.float32)[None, None, :]
    return np.ascontiguousarray(out.astype(np.float32))


def run(x, w_attn, b_attn, w_proj, b_proj, trace=False, trace_cores=None):
    in_maps, with_bias = _prep_inputs(x, w_attn, b_attn, w_proj)
    nc = _get_nc(with_bias)
    res = run_bass_kernel_spmd(
        nc, in_maps, core_ids=list(range(NCORES)),
        trace=trace, trace_cores=trace_cores,
    )
    return _combine(res.results, b_proj), res


def kernel(x, w_attn, b_attn, w_proj, b_proj):
    out, _ = run(x, w_attn, b_attn, w_proj, b_proj, trace=False)
    return out
